# revision 32
# baseline (speedup 1.0000x reference)
"""Trainium2 Bass kernel for JetGNN (2-layer SAGEConv + global mean pool).

Single fused NEFF, src-major sharding, transfer-optimized:
  - Host: graph-aligned 25600-node slabs per core; each core owns the edges
    whose SRC lies in its slab. Edges grouped by (dst supertile of 512
    global dst slots, src%4), padded to 128-edge chunks.
  - ALL per-core inputs ship as ONE int16 blob (fp16 sections via bitcast)
    to minimize per-tensor transfer/load overhead through the PJRT tunnel.
  - Device per layer: For_i over 400 supertiles: dma_gather of packed fp16
    feature rows, one-hot matmuls accumulate message partials [F, 512] in
    PSUM -> unscaled fp16 partials in supertile-major DRAM [NS*F, ST] ->
    ReduceScatter(add) gives each core its own slab's sums -> scale by
    1/deg -> dst phase: W_l @ agg + W_r @ self, bias+ReLU. Layer-2 gather
    idx derived on device (2*idx1 + gather half). Pooling on device:
    per-tile one-hot matmuls accumulated into a [64, 1024] f32 SBUF tile
    of per-(core-relative)-graph sums; single [64, 1024] fp16 output.
  - Host: transpose per-core graph sums, divide by counts, final linear.
"""

import math
import os
import threading
import time

import numpy as np

import jax

try:
    jax.config.update("jax_compilation_cache_dir",
                      "/root/.cache/jax_bass_cache")
    jax.config.update("jax_persistent_cache_min_compile_time_secs", 0)
    jax.config.update("jax_persistent_cache_min_entry_size_bytes", 0)
except Exception:
    pass

import concourse.bass as bass
import concourse.tile as tile
import concourse.mybir as mybir
from concourse import bacc
from concourse.bass_utils import run_bass_kernel_spmd

N_NODES = 200000
N_GRAPHS = 4000
N_CORES = 8
IN_CH = 32
HID = 64
SLAB = 25600
NSLOT = N_CORES * SLAB          # 204800 global dst slots
ST = 512                        # dsts per supertile
NS = NSLOT // ST                # 400 supertiles
NSC = NS // N_CORES             # supertiles per core (50)
P = 128
NT = SLAB // P                  # own dst tiles (200)
GMAX = 640                      # graphs per core bound (pool window)
PAD_GREL = 5000.0               # pool one-hot never matches

f32 = mybir.dt.float32
fp16 = mybir.dt.float16
i16 = mybir.dt.int16

GRP = [[0, 1, 2, 3, 4, 5, 6, 7]]


# ----------------------------------------------------------------- host prep
def _prep(edge_index, batch):
    src = np.asarray(edge_index[0], dtype=np.int32)
    dst = np.asarray(edge_index[1], dtype=np.int32)
    batch = np.asarray(batch, dtype=np.int32)

    gcnt = np.bincount(batch, minlength=N_GRAPHS)
    gends = np.cumsum(gcnt)
    targets = (np.arange(1, N_CORES) * N_NODES) // N_CORES
    gb = np.searchsorted(gends, targets)
    graph_bounds = np.concatenate([[0], gb + 1, [N_GRAPHS]])
    node_bounds = np.concatenate(
        [[0], gends[graph_bounds[1:-1] - 1], [N_NODES]]).astype(np.int64)
    ncounts = np.diff(node_bounds)
    # last 4 slab rows must stay zero: pad edges gather from them
    assert ncounts.max() <= SLAB - 4, ncounts.max()
    assert np.diff(graph_bounds).max() <= GMAX

    deg = np.bincount(dst, minlength=N_NODES)
    inv = (1.0 / np.maximum(deg, 1)).astype(np.float32)

    # node -> (core, local, slot) lookup tables
    node_core = np.repeat(np.arange(N_CORES, dtype=np.int32), ncounts)
    node_local = (np.arange(N_NODES, dtype=np.int32)
                  - np.repeat(node_bounds[:-1].astype(np.int32), ncounts))
    node_slot = node_core * SLAB + node_local

    cs = node_core[src]
    src_local = node_local[src]
    dslot = node_slot[dst]
    s_id = dslot >> 9
    din = (dslot & 511).astype(np.float32)
    par = src_local & 3

    # per-core inverse degree over own slab [SLAB]
    invloc = np.ones((N_CORES, SLAB), np.float16)
    for c in range(N_CORES):
        lo, hi = node_bounds[c], node_bounds[c + 1]
        invloc[c, :hi - lo] = inv[lo:hi]

    # group edges by (core, supertile, parity)
    key = (((cs * NS + s_id) << 2) | par).astype(np.int16)
    order = np.argsort(key, kind="stable")
    key_s = key[order].astype(np.int32)
    nbins = N_CORES * NS * 4
    cnt = np.bincount(key_s, minlength=nbins)
    ch_par = max(1, int(math.ceil(cnt.max() / P)))      # chunks per parity
    nch = 4 * ch_par                                     # chunks / supertile
    ngath = 2
    gsz = 2 * ch_par                                     # chunks per gather
    assert gsz * P <= 1024, gsz
    slots_pad = nch * P

    starts = np.concatenate([[0], np.cumsum(cnt)[:-1]]).astype(np.int64)
    rank = np.arange(len(src), dtype=np.int64) - starts[key_s]

    k = key_s.astype(np.int64)
    core_e = k // (NS * 4)
    s_e = (k // 4) % NS
    p_e = k & 3
    slot = ((core_e * NS + s_e) * nch + p_e * ch_par) * P + rank

    total = N_CORES * NS * slots_pad
    # pad edges gather the (zero) last x row / h1 rows and scatter into
    # dst 0 of their supertile: contribute exact zeros.
    idx1 = np.full(total, (SLAB >> 2) - 1, np.int16)
    dsti = np.zeros(total, np.int16)
    sl = src_local[order]
    idx1[slot] = (sl >> 2).astype(np.int16)
    dsti[slot] = din[order].astype(np.int16)

    # idx compact wrap: [core][16, NS * slots_pad/16]
    a = idx1.reshape(N_CORES, NS, ngath, gsz * 8, 16)
    idx1w = np.ascontiguousarray(
        a.transpose(0, 4, 1, 2, 3).reshape(N_CORES, 16, -1))

    # dst tables, value at (slot%128, chunk col):
    #   low byte  [core][128, NS * nch] u8
    #   high bits [core][128, NS] i16 (bit c = chunk c's dst >= 256)
    dsti = dsti.reshape(N_CORES, NS, nch, P)
    dvt = dsti.transpose(0, 3, 1, 2)                    # [core, P, NS, nch]
    dvlo = np.ascontiguousarray(
        (dvt & 255).astype(np.uint8).reshape(N_CORES, P, -1))
    dvhi = np.ascontiguousarray(
        ((dvt >> 8).astype(np.int16)
         << np.arange(nch, dtype=np.int16)).sum(
            axis=3, dtype=np.int16))                    # [core, P, NS]

    # pooling table: core-relative graph id per node [core][128, NT] fp16
    grelc = np.full((N_CORES, SLAB), PAD_GREL, np.float16)
    for c in range(N_CORES):
        lo, hi = node_bounds[c], node_bounds[c + 1]
        grelc[c, :hi - lo] = (batch[lo:hi] - graph_bounds[c]).astype(
            np.float16)
    grelc = np.ascontiguousarray(
        grelc.reshape(N_CORES, NT, P).transpose(0, 2, 1))

    return dict(node_bounds=node_bounds, graph_bounds=graph_bounds,
                ncounts=ncounts, gcnt=gcnt, ch_par=ch_par, nch=nch,
                ngath=ngath, gsz=gsz, idx1w=idx1w, dvlo=dvlo, dvhi=dvhi,
                invloc=invloc, grelc=grelc)


# --------------------------------------------------------------- blob layout
def _blob_layout(nch, ngath, gsz):
    icols = NS * ngath * gsz * 8
    off = {}
    cur = 0

    def sec(name, n):
        nonlocal cur
        off[name] = cur
        cur += n
        cur = (cur + P - 1) // P * P

    sec("idx", 16 * icols)
    sec("xg", SLAB * IN_CH)
    sec("dvlo", P * NS * nch // 2)     # u8 payload in i16 units
    sec("dvhi", P * NS)
    sec("inv", SLAB)
    sec("grelc", P * NT)
    sec("iota", ST)
    sec("ident", P * P)
    sec("w1lT", IN_CH * HID)
    sec("w1rT", IN_CH * HID)
    sec("w2lT", HID * HID)
    sec("w2rT", HID * HID)
    sec("b1", HID)
    sec("b2", HID)
    rows = (cur + P - 1) // P
    return off, rows


# ------------------------------------------------------------ kernel builder
def _build_nc(nch, ngath, gsz, ch_par):
    icols = NS * ngath * gsz * 8          # idx cols per 16-partition row
    ic_st = ngath * gsz * 8               # idx cols per supertile
    half = gsz * 8                        # idx cols per gather
    assert nch <= 15                      # dvhi bit word is int16

    off, rows = _blob_layout(nch, ngath, gsz)

    nc = bacc.Bacc("TRN2", target_bir_lowering=False, debug=False,
                   enable_asserts=False, num_devices=N_CORES)
    blob = nc.dram_tensor("blob", [rows, P], i16, kind="ExternalInput").ap()
    flat = blob.rearrange("r c -> (r c)")

    def sec16(name, n):
        return flat[off[name]:off[name] + n].bitcast(fp16)

    xg = sec16("xg", SLAB * IN_CH).rearrange("(r c) -> r c", c=P)
    dvlo = flat[off["dvlo"]:off["dvlo"] + P * NS * nch // 2].bitcast(
        mybir.dt.uint8).rearrange("(p c) -> p c", c=NS * nch)
    dvhi = flat[off["dvhi"]:off["dvhi"] + P * NS].rearrange(
        "(p c) -> p c", c=NS)
    inv_d = sec16("inv", SLAB).rearrange("(o n) -> o n", o=1)
    grelc_d = sec16("grelc", P * NT).rearrange("(p t) -> p t", t=NT)
    iota_d = sec16("iota", ST).rearrange("(o n) -> o n", o=1)
    ident_d = sec16("ident", P * P).rearrange("(a b) -> a b", b=P)
    w1lT_d = sec16("w1lT", IN_CH * HID).rearrange("(a b) -> a b", b=HID)
    w1rT_d = sec16("w1rT", IN_CH * HID).rearrange("(a b) -> a b", b=HID)
    w2lT_d = sec16("w2lT", HID * HID).rearrange("(a b) -> a b", b=HID)
    w2rT_d = sec16("w2rT", HID * HID).rearrange("(a b) -> a b", b=HID)
    b1_d = sec16("b1", HID).rearrange("(a b) -> a b", b=1)
    b2_d = sec16("b2", HID).rearrange("(a b) -> a b", b=1)
    idx1c = flat[off["idx"]:off["idx"] + 16 * icols].rearrange(
        "(p c) -> p c", c=icols)

    idx1r = nc.dram_tensor("idx1r", [P, icols], i16, kind="Internal").ap()
    h1d = nc.dram_tensor("h1d", [SLAB, HID], fp16, kind="Internal").ap()
    part1 = nc.dram_tensor("part1", [NS * IN_CH, ST], fp16,
                           kind="Internal").ap()
    part2 = nc.dram_tensor("part2", [NS * HID, ST], fp16,
                           kind="Internal").ap()
    cc_mode = os.environ.get("K_CC", "rs")
    rsz = N_CORES if cc_mode == "a2a" else 1
    red1 = nc.dram_tensor("red1", [rsz * NSC * IN_CH, ST], fp16,
                          kind="Internal").ap()
    red2 = nc.dram_tensor("red2", [rsz * NSC * HID, ST], fp16,
                          kind="Internal").ap()
    poolg = nc.dram_tensor("poolg", [HID, GMAX], fp16,
                           kind="ExternalOutput").ap()

    ns_run = int(os.environ.get("K_NS", NS))

    with tile.TileContext(nc) as tc:
        with tc.tile_pool(name="res", bufs=1) as rp, \
             tc.tile_pool(name="ld", bufs=3) as ld, \
             tc.tile_pool(name="g", bufs=3) as gp, \
             tc.tile_pool(name="oh", bufs=4) as ohp, \
             tc.tile_pool(name="o", bufs=3) as op_, \
             tc.tile_pool(name="st", bufs=3) as stp, \
             tc.tile_pool(name="a2ap", bufs=2) as a2ap, \
             tc.tile_pool(name="ps", bufs=2, space="PSUM") as ps, \
             tc.tile_pool(name="ps2", bufs=2, space="PSUM") as ps2, \
             tc.tile_pool(name="ps3", bufs=1, space="PSUM") as ps3:

            # ---- prologue: residents + idx replication to 128 partitions
            iota1 = rp.tile([1, ST], fp16, tag="iota1")
            nc.sync.dma_start(iota1[:], iota_d[:])
            iota_sb = rp.tile([P, ST], fp16, tag="iota")
            nc.gpsimd.partition_broadcast(iota_sb[:], iota1[:])
            ident_sb = rp.tile([P, P], fp16, tag="ident")
            nc.sync.dma_start(ident_sb[:], ident_d[:])
            grelh = rp.tile([P, NT], fp16, tag="grelh")
            nc.sync.dma_start(grelh[:], grelc_d[:])
            grel_sb = rp.tile([P, NT], f32, tag="grel")
            nc.vector.tensor_copy(grel_sb[:], grelh[:])
            h1T_res = rp.tile([HID, SLAB], fp16, tag="h1T_res")
            # h1T_res row 0 doubles as staging for the inv broadcast; it is
            # fully overwritten by layer-1 outputs afterwards.
            nc.sync.dma_start(h1T_res[0:1, :], inv_d[:])
            invb = rp.tile([HID, SLAB], fp16, tag="invb")
            nc.gpsimd.partition_broadcast(invb[:], h1T_res[0:1, :])
            w1lT = rp.tile([IN_CH, HID], fp16, tag="w1lT")
            nc.sync.dma_start(w1lT[:], w1lT_d[:])
            w1rT = rp.tile([IN_CH, HID], fp16, tag="w1rT")
            nc.sync.dma_start(w1rT[:], w1rT_d[:])
            w2lT = rp.tile([HID, HID], fp16, tag="w2lT")
            nc.sync.dma_start(w2lT[:], w2lT_d[:])
            w2rT = rp.tile([HID, HID], fp16, tag="w2rT")
            nc.sync.dma_start(w2rT[:], w2rT_d[:])
            b1h = rp.tile([HID, 1], fp16, tag="b1h")
            nc.sync.dma_start(b1h[:], b1_d[:])
            b1 = rp.tile([HID, 1], f32, tag="b1")
            nc.vector.tensor_copy(b1[:], b1h[:])
            b2h = rp.tile([HID, 1], fp16, tag="b2h")
            nc.sync.dma_start(b2h[:], b2_d[:])
            b2 = rp.tile([HID, 1], f32, tag="b2")
            nc.vector.tensor_copy(b2[:], b2h[:])
            red_sb = rp.tile([HID, SLAB], fp16, tag="red_sb")
            acc = rp.tile([HID, GMAX], f32, tag="acc")
            nc.vector.memset(acc[:], 0)

            for k in range(8):
                nc.sync.dma_start(idx1r[16 * k:16 * (k + 1), :], idx1c[:])
            tc.strict_bb_all_engine_barrier()

            pid = nc.sync.partition_id()

            idx3 = idx1r.rearrange("p (s c) -> p s c", c=ic_st)
            dv3 = dvlo.rearrange("p (s c) -> p s c", c=nch)
            dh3 = dvhi.rearrange("p (s o) -> p s o", o=1)

            def supertile_loop(lay, tabv, F, partials):
                p3 = partials.rearrange("(s f) d -> s f d", f=F)
                gslots = gsz * P
                with tc.For_i(0, ns_run) as s:
                    idx_sb = ld.tile([P, ic_st], i16, tag=f"idx{lay}")
                    nc.sync.dma_start(idx_sb[:], idx3[:, s])
                    if lay == 2:
                        idx2t = ld.tile([P, ic_st], i16, tag="idx2t")
                        for g in range(ngath):
                            nc.vector.tensor_scalar(
                                idx2t[:, g * half:(g + 1) * half],
                                idx_sb[:, g * half:(g + 1) * half],
                                2, g, op0=mybir.AluOpType.mult,
                                op1=mybir.AluOpType.add)
                        idx_sb = idx2t
                    dvh = ld.tile([P, nch], mybir.dt.uint8,
                                  tag=f"dvh{lay}")
                    nc.sync.dma_start(dvh[:], dv3[:, s])
                    hw = ld.tile([P, 1], i16, tag=f"hw{lay}")
                    nc.sync.dma_start(hw[:], dh3[:, s])
                    dvf = ld.tile([P, nch], f32, tag=f"dvf{lay}")
                    nc.vector.tensor_copy(dvf[:], dvh[:])
                    for c in range(nch):
                        hib = ld.tile([P, 1], i16, tag=f"hib{lay}")
                        nc.vector.tensor_scalar(
                            hib[:], hw[:], 1 << c, None,
                            op0=mybir.AluOpType.bitwise_and)
                        hbf = ld.tile([P, 1], f32, tag=f"hbf{lay}")
                        nc.vector.tensor_scalar(
                            hbf[:], hib[:], 0, None,
                            op0=mybir.AluOpType.is_gt)
                        nc.vector.tensor_scalar(
                            dvf[:, c:c + 1], hbf[:], 256.0, dvf[:, c:c + 1],
                            op0=mybir.AluOpType.mult,
                            op1=mybir.AluOpType.add)
                    ms = []
                    for g in range(ngath):
                        m = gp.tile([P, gsz, P], fp16, tag=f"m{lay}_{g}")
                        nc.gpsimd.dma_gather(
                            m[:], tabv,
                            idx_sb[:, g * half:(g + 1) * half],
                            gslots, gslots, P)
                        ms.append(m)
                    zp = ps.tile([F, ST], f32, tag="zp")
                    for c in range(nch):
                        par4 = min(c // ch_par, 3)
                        colblk = (par4 * IN_CH) if lay == 1 else \
                            ((par4 & 1) * HID)
                        m = ms[c // gsz]
                        oh = ohp.tile([P, ST], fp16, tag=f"oh{lay}")
                        nc.vector.tensor_scalar(
                            oh[:], iota_sb[:], dvf[:, c:c + 1], None,
                            op0=mybir.AluOpType.is_equal)
                        nc.tensor.matmul(
                            zp[:], lhsT=m[:, c % gsz, colblk:colblk + F],
                            rhs=oh[:], start=(c == 0), stop=(c == nch - 1))
                    zsb = op_.tile([F, ST], fp16, tag=f"zsb{lay}")
                    nc.vector.tensor_copy(zsb[:], zp[:])
                    nc.sync.dma_start(p3[s], zsb[:])

            xr3 = xg.rearrange("(t q) (k f) -> t (q k) f", q=P // 4, k=4)

            def dst_loop(lay, red, F, wl, wr, bb, hout_res):
                redv = red_sb[0:F, :]
                if cc_mode == "a2a":
                    r5 = red.rearrange("(j s f) d -> j f s d", j=N_CORES,
                                       f=F)
                    NQ = 25
                    SQ = NSC // NQ
                    HS = SLAB // NQ
                    rv3 = redv.rearrange("f (h s d) -> f h s d", h=NQ,
                                         d=ST)
                    for h in range(NQ):
                        nc.sync.dma_start(rv3[:, h],
                                          r5[0][:, h * SQ:(h + 1) * SQ])
                    for j in range(1, N_CORES):
                        for h in range(NQ):
                            tmp = a2ap.tile([HID, HS], fp16, tag="a2a")
                            t3 = tmp[0:F, :].rearrange(
                                "f (s d) -> f s d", d=ST)
                            nc.sync.dma_start(
                                t3, r5[j][:, h * SQ:(h + 1) * SQ])
                            nc.vector.tensor_tensor(
                                redv[:, h * HS:(h + 1) * HS],
                                redv[:, h * HS:(h + 1) * HS],
                                tmp[0:F, :], op=mybir.AluOpType.add)
                else:
                    nc.sync.dma_start(
                        redv.rearrange("f (s d) -> f s d", d=ST),
                        red.rearrange("(s f) d -> f s d", f=F))
                nc.vector.tensor_tensor(redv, redv, invb[0:F, :],
                                        op=mybir.AluOpType.mult)
                r3 = red_sb.rearrange("f (t d) -> f t d", d=P)
                gr3 = grel_sb.rearrange("p (t o) -> p t o", o=1)
                h13 = h1d.rearrange("(t d) f -> t d f", d=P)
                h1r3 = h1T_res.rearrange("f (t d) -> f t d", d=P)
                with tc.For_i(0, NT) as t:
                    if lay == 1:
                        xr = ld.tile([P, IN_CH], fp16, tag="xr")
                        nc.sync.dma_start(xr[:], xr3[t])
                        xTp = ps3.tile([IN_CH, P], fp16, tag="xTp")
                        nc.tensor.transpose(xTp[:], xr[:], ident_sb[:])
                        xT = ld.tile([IN_CH, P], fp16, tag="xT")
                        nc.vector.tensor_copy(xT[:], xTp[:])
                    z2 = ps2.tile([HID, P], f32, tag="z2")
                    nc.tensor.matmul(z2[:], lhsT=wl[:], rhs=r3[0:F, t],
                                     start=True, stop=False)
                    if lay == 1:
                        nc.tensor.matmul(z2[:], lhsT=wr[:], rhs=xT[:],
                                         start=False, stop=True)
                    else:
                        nc.tensor.matmul(z2[:], lhsT=wr[:],
                                         rhs=h1r3[:, t],
                                         start=False, stop=True)
                    hT = op_.tile([HID, P], fp16, tag=f"hT{lay}")
                    nc.scalar.activation(hT[:], z2[:],
                                         mybir.ActivationFunctionType.Relu,
                                         bias=bb[:])
                    if hout_res is not None:
                        nc.vector.tensor_copy(h1r3[:, t], hT[:])
                    tp = ps3.tile([P, HID], fp16, tag="tp")
                    nc.tensor.transpose(tp[:], hT[:],
                                        ident_sb[0:HID, 0:HID])
                    stg = stp.tile([P, HID], fp16, tag=f"stg{lay}")
                    nc.vector.tensor_copy(stg[:], tp[:])
                    if lay == 1:
                        nc.sync.dma_start(h13[t], stg[:])
                    else:
                        # pooling: accumulate graph sums into acc windows
                        g1 = stp.tile([P, 1], f32, tag="g1")
                        nc.vector.tensor_scalar(
                            g1[:], gr3[:, t], -512.0, None,
                            op0=mybir.AluOpType.add)
                        for w, (lo, wid) in enumerate(
                                [(0, ST), (ST, GMAX - ST)]):
                            ohg = stp.tile([P, wid], fp16, tag=f"ohg{w}")
                            src_g = gr3[:, t] if w == 0 else g1[:]
                            nc.vector.tensor_scalar(
                                ohg[:], iota_sb[:, 0:wid], src_g, None,
                                op0=mybir.AluOpType.is_equal)
                            pp = ps3.tile([HID, wid], f32, tag=f"pp{w}")
                            nc.tensor.matmul(pp[:], lhsT=stg[:], rhs=ohg[:],
                                             start=True, stop=True)
                            nc.vector.tensor_tensor(
                                acc[:, lo:lo + wid],
                                acc[:, lo:lo + wid], pp[:],
                                op=mybir.AluOpType.add)

            # ---- layer 1
            supertile_loop(1, xg, IN_CH, part1)
            if os.environ.get("K_NOCC"):
                nc.sync.dma_start(
                    red1[0:NSC * IN_CH, :],
                    part1.rearrange("(c r) d -> c r d",
                                    r=NSC * IN_CH)[pid])
            elif cc_mode == "a2a":
                nc.gpsimd.collective_compute(
                    kind="AllToAll", op=mybir.AluOpType.bypass,
                    replica_groups=GRP, ins=[part1], outs=[red1])
            else:
                nc.gpsimd.collective_compute(
                    kind="ReduceScatter", op=mybir.AluOpType.add,
                    replica_groups=GRP, ins=[part1], outs=[red1])
            dst_loop(1, red1, IN_CH, w1lT, w1rT, b1, h1T_res)

            # pad edges gather h1d rows SLAB-4..SLAB-1: force them to zero
            # (h1 there is already 0 when b1 == 0, but don't rely on it)
            zt = stp.tile([4, HID], fp16, tag="zt")
            nc.vector.memset(zt[:], 0)
            nc.sync.dma_start(h1d[SLAB - 4:SLAB, :], zt[:])

            # ---- layer 2
            h1v = h1d.rearrange("(r k) f -> r (k f)", k=2)
            supertile_loop(2, h1v, HID, part2)
            if os.environ.get("K_NOCC"):
                nc.sync.dma_start(
                    red2[0:NSC * HID, :],
                    part2.rearrange("(c r) d -> c r d",
                                    r=NSC * HID)[pid])
            elif cc_mode == "a2a":
                nc.gpsimd.collective_compute(
                    kind="AllToAll", op=mybir.AluOpType.bypass,
                    replica_groups=GRP, ins=[part2], outs=[red2])
            else:
                nc.gpsimd.collective_compute(
                    kind="ReduceScatter", op=mybir.AluOpType.add,
                    replica_groups=GRP, ins=[part2], outs=[red2])
            dst_loop(2, red2, HID, w2lT, w2rT, b2, None)

            # ---- write pooled graph sums
            poolg_sb = rp.tile([HID, GMAX], fp16, tag="poolg_sb")
            nc.vector.tensor_copy(poolg_sb[:], acc[:])
            nc.sync.dma_start(poolg[:], poolg_sb[:])

    nc.compile()
    return nc


_NC_CACHE = {}


def _make_blob(pp, x, W1_l, W1_r, W2_l, W2_r, b1, b2):
    nch, ngath, gsz = pp["nch"], pp["ngath"], pp["gsz"]
    off, rows = _blob_layout(nch, ngath, gsz)
    nb = pp["node_bounds"]

    def put16(blob, name, arr):
        a = np.ascontiguousarray(arr, dtype=np.float16).reshape(-1)
        blob[off[name]:off[name] + a.size] = a.view(np.int16)

    com = np.zeros(rows * P, np.int16)
    comf = {
        "iota": np.arange(ST, dtype=np.float16),
        "ident": np.eye(P, dtype=np.float16),
        "w1lT": np.asarray(W1_l).T.astype(np.float16),
        "w1rT": np.asarray(W1_r).T.astype(np.float16),
        "w2lT": np.asarray(W2_l).T.astype(np.float16),
        "w2rT": np.asarray(W2_r).T.astype(np.float16),
        "b1": np.asarray(b1, np.float16),
        "b2": np.asarray(b2, np.float16),
    }
    for k, v in comf.items():
        put16(com, k, v)

    blobs = []
    for c in range(N_CORES):
        blob = com.copy()
        lo, hi = nb[c], nb[c + 1]
        xs = np.zeros((SLAB, IN_CH), np.float16)
        xs[:hi - lo] = x[lo:hi].astype(np.float16)
        put16(blob, "xg", xs)
        lo8 = pp["dvlo"][c].reshape(-1)
        blob[off["dvlo"]:off["dvlo"] + lo8.size // 2] = lo8.view(np.int16)
        hi = pp["dvhi"][c].reshape(-1)
        blob[off["dvhi"]:off["dvhi"] + hi.size] = hi
        put16(blob, "inv", pp["invloc"][c])
        put16(blob, "grelc", pp["grelc"][c])
        ib = pp["idx1w"][c].reshape(-1)
        blob[off["idx"]:off["idx"] + ib.size] = ib
        blobs.append(blob.reshape(rows, P))
    return blobs


def kernel(x, edge_index, batch, W1_l, b1, W1_r, W2_l, b2, W2_r, W_lin,
           b_lin, _timing=None):
    x = np.asarray(x, dtype=np.float32)
    batch_np = np.asarray(batch, dtype=np.int64)

    t0 = time.time()
    # speculatively build the expected-config NEFF while prep runs
    exp_key = (12, 2, 6, 3)
    th = None
    if exp_key not in _NC_CACHE:
        def _bg():
            try:
                _NC_CACHE[exp_key] = _build_nc(*exp_key)
            except Exception:
                pass
        th = threading.Thread(target=_bg)
        th.start()
    pp = _prep(edge_index, batch_np)
    t_prep = time.time() - t0

    t0 = time.time()
    if th is not None:
        th.join()
    key = (pp["nch"], pp["ngath"], pp["gsz"], pp["ch_par"])
    if key not in _NC_CACHE:
        _NC_CACHE[key] = _build_nc(*key)
    nc = _NC_CACHE[key]
    t_build = time.time() - t0

    blobs = _make_blob(pp, x, W1_l, W1_r, W2_l, W2_r, b1, b2)
    in_maps = [dict(blob=b) for b in blobs]

    t0 = time.time()
    res = None
    for attempt in range(3):
        try:
            res = run_bass_kernel_spmd(nc, in_maps,
                                       core_ids=list(range(N_CORES)))
        except Exception:
            if attempt == 2:
                raise
            continue
        ok = True
        for r in res.results:
            pg = r["poolg"]
            # post-stall corruption returns NaN/Inf or silent all-zeros
            if not np.isfinite(pg).all() or not pg.any():
                ok = False
                break
        if ok:
            break
    t_run = time.time() - t0

    # ---- host: per-core graph sums -> mean pool -> final linear
    t0 = time.time()
    gb = pp["graph_bounds"]
    pool = np.zeros((N_GRAPHS, HID), np.float32)
    for c in range(N_CORES):
        ng = gb[c + 1] - gb[c]
        pool[gb[c]:gb[c + 1]] = res.results[c]["poolg"][:, :ng].T
    cnt = np.maximum(pp["gcnt"], 1).astype(np.float32)
    pooled = pool / cnt[:, None]
    out = (pooled @ np.asarray(W_lin, np.float32).T
           + np.asarray(b_lin, np.float32)).astype(np.float32)
    t_host = time.time() - t0

    if _timing is not None:
        _timing.update(dtA=t_run, dtB=0.0, prep=t_prep, build=t_build,
                       host=t_host)
    return out
